# revision 6
# baseline (speedup 1.0000x reference)
"""TRN2 Bass kernel for nn_BeyazKusAIAttention_36515811951168.

Key reduction: the reference applies softmax over a size-1 axis, which is
identically 1.0, so attention weights are exactly 1 and the module collapses
to
    y = (x @ Wv^T) @ Wfold^T,  with  Wfold = Wo.reshape(4096,4,1024).sum(1)
(q/rope/scores/mask are dead code; `out` is v tiled over the 4 heads, and the
o-projection of the tiled v folds head-wise into Wfold).  5x FLOP reduction
vs the reference graph; both matmuls run in fp16 with fp32 PSUM accumulation
(measured end-to-end rel err ~3.9e-4 vs the fp32 reference).

Execution: data-parallel over the 16384 = batch*seq rows across 8 NeuronCores
(no collectives).  Per-core two-phase program (R = 2048 rows):

Phase 1 (MM1, k-outer): for each 512-row chunk, v^T = Wv @ x^T accumulated
  over K=4096 into all 8 PSUM banks (one per 128-wide v-col tile), then each
  bank is evicted to a persistent SBUF vs tile (fp16).  k-outer ordering
  paces PE consumption of the wv/xt DMA stream at ~230 GB/s < HBM rate, so
  even the cold start (nothing resident) never idles the PE for more than
  one tile's DMA latency.  Weights Wv^T (8MB fp16) stay SBUF-resident.
Phase 2 (MM2): y = v @ Wfold^T with vs tiles stationary and SBUF-resident
  Wfold^T (8MB fp16) moving; 8-long K-accumulation chains, one PSUM bank
  per 512-col output group with 8-bank rotation, DVE eviction, DMA out.
  Wfold^T's 8MB load is emitted LAST into the FIFO DMA queue so it hides
  behind the whole phase-1 stream instead of blocking it (it is only
  needed ~160us in).

PE instruction stream is 2048 N=512 fp16 matmuls (the minimum: PSUM banks
cap fp32 matmul output at 512 columns) and runs back-to-back at the PE
streaming rate; measured ~383us on 8 cores (PE roofline for 1.048M
PE-cycles/core is ~437us at the nominal 2.4 GHz clock).

Host-side layouts (partition dim = contraction dim for both matmuls):
  xt [32,128,R]: xt[k,p,r] = x[row r, dim 128k+p]     (transposed shard)
  wvt[32,128,1024]: wvt[k,p,m] = Wv[m, 128k+p]
  wft[8,128,4096]:  wft[k,p,n] = Wfold[n, 128k+p]
  y  [R/128,128,4096]: y[t,p,n] = out[row 128t+p, n]
"""
import numpy as np
import concourse.bass as bass
from concourse import bacc
import concourse.mybir as mybir
from concourse.tile import TileContext
from concourse.bass_utils import run_bass_kernel_spmd

DIM = 4096
KV = 1024
N_CORES = 8
ROWS_TOTAL = 4 * 4096
ROWS = ROWS_TOTAL // N_CORES   # 2048
KT1 = DIM // 128               # 32 k-tiles, MM1
MT1 = KV // 128                # 8 m-tiles (v cols)
KT2 = KV // 128                # 8 k-tiles, MM2
NC2 = DIM // 512               # 8 y-col chunks
CH = 512

_nc_cache = {}


def _build(rows=ROWS, loop_n=1):
    """loop_n > 1 wraps the program in a hardware loop — used only by the
    loop-slope timing harness (test.py), never by kernel()."""
    nch = rows // CH
    f32 = mybir.dt.float32
    f16 = mybir.dt.float16

    nc = bacc.Bacc(None, target_bir_lowering=False)
    XT = nc.dram_tensor("xt", [KT1, 128, rows], f16, kind="ExternalInput")
    WVT = nc.dram_tensor("wvt", [KT1, 128, KV], f16, kind="ExternalInput")
    WFT = nc.dram_tensor("wft", [KT2, 128, DIM], f16, kind="ExternalInput")
    Y = nc.dram_tensor("y", [rows // 128, 128, DIM], f32,
                       kind="ExternalOutput")

    with TileContext(nc) as tc:
        with (
            tc.tile_pool(name="wf", bufs=1) as wfpool,
            tc.tile_pool(name="wv", bufs=1) as wvpool,
            tc.tile_pool(name="xts", bufs=1) as xtpool,
            tc.tile_pool(name="vss", bufs=1) as vspool,
            tc.tile_pool(name="yst", bufs=4) as ypool,
            tc.tile_pool(name="ps", bufs=1, space="PSUM") as pspool,
        ):
            def body():
                # PE warmup: dummy matmuls on memset tiles fill the
                # ~2.5-3.5us wait for the first wv/xt DMA, so the HAM
                # clock-gate is already at 8/8 (2.4GHz) when the real
                # stream starts instead of ramping through it.
                wrm_w = wvpool.tile([128, 128], f16, tag="wrm_w")
                nc.vector.memset(wrm_w[:], 0.0)
                wrm_x = wvpool.tile([128, 256], f16, tag="wrm_x")
                nc.vector.memset(wrm_x[:], 0.0)
                wrm_p = pspool.tile([128, 256], f32, tag="b0",
                                    name="wrm_p")
                for i in range(12):
                    nc.tensor.matmul(wrm_p[:], wrm_w[:], wrm_x[:],
                                     start=True, stop=True)
                # wv + chunk-0 xt interleaved: chunk 0's k-loop consumes
                # them pairwise straight off the DMA stream.
                wv = []
                xt = [None] * KT1
                for k in range(KT1):
                    wvk = wvpool.tile([128, KV], f16, tag=f"wv{k}")
                    nc.sync.dma_start(wvk[:], WVT[k])
                    wv.append(wvk)
                    xtk = xtpool.tile([128, CH], f16, tag=f"xt{k}")
                    nc.sync.dma_start(xtk[:], XT[k, :, 0:CH])
                    xt[k] = xtk
                # chunk-1 xt queued before the (much later needed) wf load:
                # the DMA queue is FIFO, so wf's 8MB must not block chunk 1.
                xt_next = [None] * KT1
                for k in range(KT1):
                    xtk = xtpool.tile([128, CH], f16, tag=f"xt{k}")
                    nc.sync.dma_start(xtk[:], XT[k, :, CH:2 * CH])
                    xt_next[k] = xtk
                # wf as 64 separate [128,512] tiles: every MM2 moving
                # operand is then a whole tile (zero-offset AP), measured
                # ~3% faster than 3D-sliced APs of [128,8,512] tiles.
                wf = []
                for n in range(NC2):
                    row = []
                    for k in range(KT2):
                        wfnk = wfpool.tile([128, 512], f16,
                                           tag=f"wf{n}_{k}",
                                           name=f"wf_{n}_{k}")
                        nc.sync.dma_start(wfnk[:],
                                          WFT[k, :, n * 512:(n + 1) * 512])
                        row.append(wfnk)
                    wf.append(row)

                # ---- phase 1: v^T = Wv @ x^T, k-outer, 8 banks ----
                vs = [[None] * MT1 for _ in range(nch)]
                for rc in range(nch):
                    if rc == 1:
                        xt = xt_next
                    elif rc > 1:
                        for k in range(KT1):
                            xtk = xtpool.tile([128, CH], f16, tag=f"xt{k}")
                            nc.sync.dma_start(
                                xtk[:], XT[k, :, rc * CH:(rc + 1) * CH])
                            xt[k] = xtk
                    ps1 = [pspool.tile([128, CH], f32, tag=f"b{m}",
                                       name=f"ps1_{rc}_{m}")
                           for m in range(MT1)]
                    for k in range(KT1):
                        for m in range(MT1):
                            nc.tensor.matmul(
                                ps1[m][:], wv[k][:, m * 128:(m + 1) * 128],
                                xt[k][:], start=(k == 0), stop=(k == KT1 - 1))
                    for m in range(MT1):
                        v = vspool.tile([128, CH], f16, tag=f"vs{rc}_{m}")
                        nc.vector.tensor_copy(v[:], ps1[m][:])
                        vs[rc][m] = v

                # ---- phase 2: y = v @ Wfold^T ----
                g = 0
                for rc in range(nch):
                    for sub in range(CH // 128):
                        for n in range(NC2):
                            ps2 = pspool.tile(
                                [128, 512], f32, tag=f"b{g % 8}",
                                name=f"ps2_{rc}_{sub}_{n}")
                            g += 1
                            for k2 in range(KT2):
                                nc.tensor.matmul(
                                    ps2[:],
                                    vs[rc][k2][:, sub * 128:(sub + 1) * 128],
                                    wf[n][k2][:],
                                    start=(k2 == 0), stop=(k2 == KT2 - 1))
                            ys = ypool.tile([128, 512], f32, tag="ys")
                            nc.vector.tensor_copy(ys[:], ps2[:])
                            nc.sync.dma_start(
                                Y[rc * (CH // 128) + sub, :,
                                  n * 512:(n + 1) * 512], ys[:])
            if loop_n == 1:
                body()
            else:
                with tc.For_i(0, loop_n):
                    body()
    nc.compile()
    return nc


def prep_inputs(x, Wv, Wo, n_cores=N_CORES):
    """Host-side relayout: transpose x once, fold Wo over heads, cast to
    fp16; returns per-core input maps."""
    x = np.asarray(x)
    Wv = np.asarray(Wv, dtype=np.float32)
    Wo = np.asarray(Wo, dtype=np.float32)
    x2 = np.ascontiguousarray(
        x.reshape(ROWS_TOTAL, DIM).T).astype(np.float16)
    xt_all = x2.reshape(KT1, 128, ROWS_TOTAL)
    wvt = np.ascontiguousarray(Wv.T).astype(np.float16).reshape(KT1, 128, KV)
    wfold = Wo.reshape(DIM, 4, KV).sum(axis=1)
    wft = np.ascontiguousarray(wfold.T).astype(np.float16).reshape(
        KT2, 128, DIM)
    in_maps = []
    for c in range(n_cores):
        in_maps.append({
            "xt": np.ascontiguousarray(xt_all[:, :, c * ROWS:(c + 1) * ROWS]),
            "wvt": wvt,
            "wft": wft,
        })
    return in_maps


def kernel(x, Wq, Wk, Wv, Wo, mask):
    B, S, D = np.asarray(x).shape
    assert D == DIM and B * S == ROWS_TOTAL
    in_maps = prep_inputs(x, Wv, Wo)
    if "nc" not in _nc_cache:
        _nc_cache["nc"] = _build()
    nc = _nc_cache["nc"]

    # transient NRT device errors (e.g. NRT_EXEC_UNIT_UNRECOVERABLE right
    # after another process released the cores) succeed on retry
    last_err = None
    for _attempt in range(3):
        try:
            results = run_bass_kernel_spmd(
                nc, in_maps, core_ids=list(range(N_CORES))).results
            break
        except Exception as e:  # noqa: BLE001
            last_err = e
    else:
        raise last_err
    shards = [r["y"].reshape(ROWS, DIM) for r in results]
    out = np.concatenate(shards, axis=0).reshape(B, S, DIM)
    return out.astype(np.float32, copy=False)


# revision 7
# speedup vs baseline: 1.0083x; 1.0083x over previous
"""TRN2 Bass kernel for nn_BeyazKusAIAttention_36515811951168.

Key reduction: the reference applies softmax over a size-1 axis, which is
identically 1.0, so attention weights are exactly 1 and the module collapses
to
    y = (x @ Wv^T) @ Wfold^T,  with  Wfold = Wo.reshape(4096,4,1024).sum(1)
(q/rope/scores/mask are dead code; `out` is v tiled over the 4 heads, and the
o-projection of the tiled v folds head-wise into Wfold).  5x FLOP reduction
vs the reference graph; both matmuls run in fp16 with fp32 PSUM accumulation
(measured end-to-end rel err ~3.9e-4 vs the fp32 reference).

Execution: data-parallel over the 16384 = batch*seq rows across 8 NeuronCores
(no collectives).  Per-core two-phase program (R = 2048 rows):

Phase 1 (MM1, k-outer): for each 512-row chunk, v^T = Wv @ x^T accumulated
  over K=4096 into all 8 PSUM banks (one per 128-wide v-col tile), then each
  bank is evicted to a persistent SBUF vs tile (fp16).  k-outer ordering
  paces PE consumption of the wv/xt DMA stream at ~230 GB/s < HBM rate, so
  even the cold start (nothing resident) never idles the PE for more than
  one tile's DMA latency.  Weights Wv^T (8MB fp16) stay SBUF-resident.
Phase 2 (MM2): y = v @ Wfold^T with vs tiles stationary and SBUF-resident
  Wfold^T (8MB fp16) moving; 8-long K-accumulation chains, one PSUM bank
  per 512-col output group with 8-bank rotation, DVE eviction, DMA out.
  Wfold^T's 8MB load is emitted LAST into the FIFO DMA queue so it hides
  behind the whole phase-1 stream instead of blocking it (it is only
  needed ~160us in).

PE instruction stream is 2048 N=512 fp16 matmuls (the minimum: PSUM banks
cap fp32 matmul output at 512 columns) and runs back-to-back at the PE
streaming rate; measured ~383us on 8 cores (PE roofline for 1.048M
PE-cycles/core is ~437us at the nominal 2.4 GHz clock).

Host-side layouts (partition dim = contraction dim for both matmuls):
  xt [32,128,R]: xt[k,p,r] = x[row r, dim 128k+p]     (transposed shard)
  wvt[32,128,1024]: wvt[k,p,m] = Wv[m, 128k+p]
  wft[8,128,4096]:  wft[k,p,n] = Wfold[n, 128k+p]
  y  [R/128,128,4096]: y[t,p,n] = out[row 128t+p, n]
"""
import numpy as np
import concourse.bass as bass
from concourse import bacc
import concourse.mybir as mybir
from concourse.tile import TileContext
from concourse.bass_utils import run_bass_kernel_spmd

DIM = 4096
KV = 1024
N_CORES = 8
ROWS_TOTAL = 4 * 4096
ROWS = ROWS_TOTAL // N_CORES   # 2048
KT1 = DIM // 128               # 32 k-tiles, MM1
MT1 = KV // 128                # 8 m-tiles (v cols)
KT2 = KV // 128                # 8 k-tiles, MM2
NC2 = DIM // 512               # 8 y-col chunks
CH = 512

_nc_cache = {}


def _build(rows=ROWS, loop_n=1):
    """loop_n > 1 wraps the program in a hardware loop — used only by the
    loop-slope timing harness (test.py), never by kernel()."""
    nch = rows // CH
    f32 = mybir.dt.float32
    f16 = mybir.dt.float16

    nc = bacc.Bacc(None, target_bir_lowering=False)
    XT = nc.dram_tensor("xt", [KT1, 128, rows], f16, kind="ExternalInput")
    WVT = nc.dram_tensor("wvt", [KT1, 128, KV], f16, kind="ExternalInput")
    WFT = nc.dram_tensor("wft", [KT2, 128, DIM], f16, kind="ExternalInput")
    Y = nc.dram_tensor("y", [rows // 128, 128, DIM], f32,
                       kind="ExternalOutput")

    with TileContext(nc) as tc:
        with (
            tc.tile_pool(name="wf", bufs=1) as wfpool,
            tc.tile_pool(name="wv", bufs=1) as wvpool,
            tc.tile_pool(name="xts", bufs=1) as xtpool,
            tc.tile_pool(name="vss", bufs=1) as vspool,
            tc.tile_pool(name="yst", bufs=4) as ypool,
            tc.tile_pool(name="ps", bufs=1, space="PSUM") as pspool,
        ):
            def body():
                # PE warmup: dummy matmuls on memset tiles fill the
                # ~2.5-3.5us wait for the first wv/xt DMA, so the HAM
                # clock-gate is already at 8/8 (2.4GHz) when the real
                # stream starts instead of ramping through it.
                wrm_w = wvpool.tile([128, 128], f16, tag="wrm_w")
                nc.vector.memset(wrm_w[:], 0.0)
                wrm_x = wvpool.tile([128, 256], f16, tag="wrm_x")
                nc.vector.memset(wrm_x[:], 0.0)
                wrm_p = pspool.tile([128, 256], f32, tag="b0",
                                    name="wrm_p")
                for i in range(12):
                    nc.tensor.matmul(wrm_p[:], wrm_w[:], wrm_x[:],
                                     start=True, stop=True)
                # wv + chunk-0 xt interleaved: chunk 0's k-loop consumes
                # them pairwise straight off the DMA stream.
                wv = []
                xt = [None] * KT1
                for k in range(KT1):
                    wvk = wvpool.tile([128, KV], f16, tag=f"wv{k}")
                    nc.sync.dma_start(wvk[:], WVT[k])
                    wv.append(wvk)
                    xtk = xtpool.tile([128, CH], f16, tag=f"xt{k}")
                    nc.sync.dma_start(xtk[:], XT[k, :, 0:CH])
                    xt[k] = xtk
                # chunk-1 xt queued before the (much later needed) wf load:
                # the DMA queue is FIFO, so wf's 8MB must not block chunk 1.
                xt_next = [None] * KT1
                for k in range(KT1):
                    xtk = xtpool.tile([128, CH], f16, tag=f"xt{k}")
                    nc.sync.dma_start(xtk[:], XT[k, :, CH:2 * CH])
                    xt_next[k] = xtk
                # wf as 64 separate [128,512] tiles: every MM2 moving
                # operand is then a whole tile (zero-offset AP), measured
                # ~3% faster than 3D-sliced APs of [128,8,512] tiles.
                wf = []
                for n in range(NC2):
                    row = []
                    for k in range(KT2):
                        wfnk = wfpool.tile([128, 512], f16,
                                           tag=f"wf{n}_{k}",
                                           name=f"wf_{n}_{k}")
                        nc.sync.dma_start(wfnk[:],
                                          WFT[k, :, n * 512:(n + 1) * 512])
                        row.append(wfnk)
                    wf.append(row)

                # ---- phase 1: v^T = Wv @ x^T, k-outer, 8 banks ----
                vs = [[None] * MT1 for _ in range(nch)]
                for rc in range(nch):
                    if rc == 1:
                        xt = xt_next
                    elif rc > 1:
                        for k in range(KT1):
                            xtk = xtpool.tile([128, CH], f16, tag=f"xt{k}")
                            nc.sync.dma_start(
                                xtk[:], XT[k, :, rc * CH:(rc + 1) * CH])
                            xt[k] = xtk
                    ps1 = [pspool.tile([128, CH], f32, tag=f"b{m}",
                                       name=f"ps1_{rc}_{m}")
                           for m in range(MT1)]
                    for k in range(KT1):
                        for m in range(MT1):
                            nc.tensor.matmul(
                                ps1[m][:], wv[k][:, m * 128:(m + 1) * 128],
                                xt[k][:], start=(k == 0), stop=(k == KT1 - 1))
                    for m in range(MT1):
                        v = vspool.tile([128, CH], f16, tag=f"vs{rc}_{m}")
                        nc.vector.tensor_copy(v[:], ps1[m][:])
                        vs[rc][m] = v

                # ---- phase 2: y = v @ Wfold^T ----
                g = 0
                for rc in range(nch):
                    for sub in range(CH // 128):
                        for n in range(NC2):
                            ps2 = pspool.tile(
                                [128, 512], f32, tag=f"b{g % 8}",
                                name=f"ps2_{rc}_{sub}_{n}")
                            g += 1
                            for k2 in range(KT2):
                                nc.tensor.matmul(
                                    ps2[:],
                                    vs[rc][k2][:, sub * 128:(sub + 1) * 128],
                                    wf[n][k2][:],
                                    start=(k2 == 0), stop=(k2 == KT2 - 1))
                            ys = ypool.tile([128, 512], f32, tag="ys")
                            nc.vector.tensor_copy(ys[:], ps2[:])
                            nc.sync.dma_start(
                                Y[rc * (CH // 128) + sub, :,
                                  n * 512:(n + 1) * 512], ys[:])
            if loop_n == 1:
                body()
            else:
                with tc.For_i(0, loop_n):
                    body()
    nc.compile()
    return nc


def prep_inputs(x, Wv, Wo, n_cores=N_CORES):
    """Host-side relayout: transpose x once, fold Wo over heads, cast to
    fp16; returns per-core input maps."""
    x = np.asarray(x)
    Wv = np.asarray(Wv, dtype=np.float32)
    Wo = np.asarray(Wo, dtype=np.float32)
    # cast to fp16 BEFORE transposing: halves the bytes moved by the
    # 256MB transpose (values identical - elementwise cast commutes)
    x16 = x.reshape(ROWS_TOTAL, DIM).astype(np.float16)
    x2 = np.ascontiguousarray(x16.T)
    xt_all = x2.reshape(KT1, 128, ROWS_TOTAL)
    wvt = np.ascontiguousarray(Wv.T).astype(np.float16).reshape(KT1, 128, KV)
    wfold = Wo.reshape(DIM, 4, KV).sum(axis=1)
    wft = np.ascontiguousarray(wfold.T).astype(np.float16).reshape(
        KT2, 128, DIM)
    in_maps = []
    for c in range(n_cores):
        in_maps.append({
            "xt": np.ascontiguousarray(xt_all[:, :, c * ROWS:(c + 1) * ROWS]),
            "wvt": wvt,
            "wft": wft,
        })
    return in_maps


def kernel(x, Wq, Wk, Wv, Wo, mask):
    B, S, D = np.asarray(x).shape
    assert D == DIM and B * S == ROWS_TOTAL
    in_maps = prep_inputs(x, Wv, Wo)
    if "nc" not in _nc_cache:
        _nc_cache["nc"] = _build()
    nc = _nc_cache["nc"]

    # transient NRT device errors (e.g. NRT_EXEC_UNIT_UNRECOVERABLE right
    # after another process released the cores) succeed on retry
    last_err = None
    for _attempt in range(3):
        try:
            results = run_bass_kernel_spmd(
                nc, in_maps, core_ids=list(range(N_CORES))).results
            break
        except Exception as e:  # noqa: BLE001
            last_err = e
    else:
        raise last_err
    shards = [r["y"].reshape(ROWS, DIM) for r in results]
    out = np.concatenate(shards, axis=0).reshape(B, S, DIM)
    return out.astype(np.float32, copy=False)
